# revision 19
# baseline (speedup 1.0000x reference)
"""CrossNetwork (DCN) kernel for 8 Trainium2 NeuronCores.

Math: the L=4 cross layers  x_{i+1} = x0 * (x_i . w_i) + b_i + x_i  collapse to
    out = alpha * x0 + beta
where beta = sum_l b_l, and per-row alpha follows the scalar recurrence
    t_l = x0 . w_l          (4 per-row dot products)
    u_l = 1 + t_l
    alpha = u_0;  alpha = alpha * u_l + c_l   (l = 1..3)
with c_l = (sum_{j<l} b_j) . w_l  (host-precomputed scalars).

The per-row dots run on the tensor engine in float32r (fp32 data, fast PE
mode). Per 512-row supertile: the 32 [128,128] chunks are PE-transposed into
PSUM, bounced to SBUF by the scalar engine, then contracted against W^T in 8
column-packed matmuls (tile_position col groups, 4 concurrent each) that
accumulate a partial T^T per col group. A tiny selection-matrix matmul per row
block combines the col groups and transposes T^T back to [rows, l] in one op.
The finale runs as one fused DVE op per row block: out = (x * alpha) + beta.

The per-supertile tail (selection matmuls, alpha recurrence, finale, store) is
emitted one supertile late so its PE ops never block the next supertile's
transposes in the in-order PE FIFO.

Sharding: data-parallel over the batch dim, 4096 rows per core; the tiny
weights/biases-derived tensors are replicated.
"""

import sys

if "/opt/trn_rl_repo" not in sys.path:
    sys.path.insert(0, "/opt/trn_rl_repo")

from contextlib import ExitStack

import numpy as np

import concourse.bass as bass
import concourse.tile as tile
from concourse import bacc, mybir
from concourse.bass_utils import run_bass_kernel_spmd
from concourse.masks import make_identity

N_CORES = 8
B, D, L = 32768, 1024, 4
R = B // N_CORES          # rows per core
P = 128                   # partitions
SB = 4                    # row-blocks per supertile (512 rows)
NST = R // (SB * P)       # supertiles per core
NCH = D // P              # 128-col chunks per row
NGRP = 4                  # tile_position col groups for packed dot matmuls
F32R = mybir.dt.float32r


def build_program(rows=R):
    nst = rows // (SB * P)
    nc = bacc.Bacc("TRN2", target_bir_lowering=False, debug=False)
    x = nc.dram_tensor("x", [rows, D], F32R, kind="ExternalInput")
    wt = nc.dram_tensor("wt", [D, L], F32R, kind="ExternalInput")
    iden = nc.dram_tensor("iden", [P, P], F32R, kind="ExternalInput")
    beta = nc.dram_tensor("beta", [1, D], mybir.dt.float32, kind="ExternalInput")
    cvec = nc.dram_tensor("cvec", [1, L], mybir.dt.float32, kind="ExternalInput")
    out = nc.dram_tensor("out", [rows, D], mybir.dt.float32, kind="ExternalOutput")

    xr = x.rearrange("(s b p) d -> s p b d", b=SB, p=P)
    outr = out.rearrange("(s b p) d -> s p b d", b=SB, p=P)

    with tile.TileContext(nc) as tc, ExitStack() as ctx:
        consts = ctx.enter_context(tc.tile_pool(name="consts", bufs=1))
        xf_pool = ctx.enter_context(tc.tile_pool(name="xf", bufs=6))
        xbt_sb_pool = ctx.enter_context(tc.tile_pool(name="xbt_sb", bufs=10))
        small_pool = ctx.enter_context(tc.tile_pool(name="small", bufs=3))
        osb_pool = ctx.enter_context(tc.tile_pool(name="osb", bufs=3))
        xbt_ps_pool = ctx.enter_context(tc.tile_pool(name="xbt_ps", bufs=4, space="PSUM"))
        tt_ps_pool = ctx.enter_context(tc.tile_pool(name="tt_ps", bufs=2, space="PSUM"))
        t_ps_pool = ctx.enter_context(tc.tile_pool(name="t_ps", bufs=2, space="PSUM"))

        # constants
        ident = consts.tile([P, P], F32R)
        nc.gpsimd.dma_start(out=ident, in_=iden[:])
        ident4 = consts.tile([L, L], mybir.dt.float32)
        make_identity(nc, ident4)
        wt_sb = consts.tile([P, NCH, L], F32R)
        nc.gpsimd.dma_start(out=wt_sb, in_=wt.rearrange("(k p) l -> p k l", p=P))
        beta_sb = consts.tile([P, D], mybir.dt.float32)
        beta_ap = beta[:]
        nc.gpsimd.dma_start(
            out=beta_sb,
            in_=bass.AP(
                tensor=beta_ap.tensor,
                offset=beta_ap.offset,
                ap=[[0, P], [1, D]],
            ),
        )
        c_sb = consts.tile([P, L], mybir.dt.float32)
        cvec_ap = cvec[:]
        nc.gpsimd.dma_start(
            out=c_sb,
            in_=bass.AP(
                tensor=cvec_ap.tensor,
                offset=cvec_ap.offset,
                ap=[[0, P], [1, L]],
            ),
        )

        def make_tail(st, xf_t, tt_s):
            # Everything past the dot matmuls for supertile `st`; emitted one
            # supertile late (see module docstring).
            def tail():
                # transpose T^T back to [rows, l] per row-block
                t_p = t_ps_pool.tile([P, SB, L], mybir.dt.float32)
                for b in range(SB):
                    nc.tensor.transpose(
                        t_p[:, b], tt_s[:, b * P:(b + 1) * P], ident4
                    )
                u_t = small_pool.tile([P, SB, L], mybir.dt.float32, tag="u")
                # u = 1 + t on ACT (psum -> sbuf with the +1 fused); keeping
                # this off DVE frees t_p fast even when DVE is deep in affines
                nc.scalar.activation(
                    u_t, t_p, mybir.ActivationFunctionType.Identity, bias=1.0
                )

                # per row block: alpha via one linear scan over layers
                # (state = u_0; state = state*u_l + c_l), then the fused
                # finale out = (x * alpha) + beta, then store per pair
                al = small_pool.tile([P, SB, L], mybir.dt.float32, tag="al")
                for b in range(SB):
                    nc.vector.tensor_tensor_scan(
                        out=al[:, b],
                        data0=u_t[:, b],
                        data1=c_sb,
                        initial=1.0,
                        op0=mybir.AluOpType.mult,
                        op1=mybir.AluOpType.add,
                    )
                o_t = osb_pool.tile([P, SB, D], mybir.dt.float32)
                for b in range(SB):
                    nc.vector.affine_then_add(
                        out=o_t[:, b],
                        in0=xf_t[:, b],
                        in1=beta_sb,
                        scale=al[:, b, L - 1:L],
                        bias=0.0,
                    )
                    if b % 2 == 1:
                        # gpsimd (SWDGE) trigger: keeps the store's wait on
                        # the DVE affines off the ACT and Sync queues
                        nc.gpsimd.dma_start(
                            out=outr[st][:, b - 1:b + 1],
                            in_=o_t[:, b - 1:b + 1],
                        )

            return tail

        pending_tail = None
        for st in range(nst):
            xf_t = xf_pool.tile([P, SB, D], F32R)
            nc.sync.dma_start(out=xf_t, in_=xr[st])

            # transpose all 32 [128,128] chunks, bouncing each column chunk
            # through PSUM to SBUF
            xbt_list = []
            for c in range(NCH):
                cs = slice(c * P, (c + 1) * P)
                xbt_p = xbt_ps_pool.tile([P, SB, P], F32R)
                for b in range(SB):
                    nc.tensor.transpose(xbt_p[:, b], xf_t[:, b, cs], ident)
                xbt_s = xbt_sb_pool.tile([P, SB * P], F32R)
                nc.scalar.copy(xbt_s, xbt_p)
                xbt_list.append(xbt_s)

            # 8 dot matmuls accumulating T^T[l, rows] over column chunks
            tt_p = tt_ps_pool.tile([L, SB * P], mybir.dt.float32)
            for c in range(NCH):
                nc.tensor.matmul(
                    tt_p,
                    wt_sb[:, c],
                    xbt_list[c],
                    start=(c == 0),
                    stop=(c == NCH - 1),
                )

            tt_s = small_pool.tile([L, SB * P], mybir.dt.float32, tag="tt_s")
            nc.scalar.copy(tt_s, tt_p)

            if pending_tail is not None:
                pending_tail()
            pending_tail = make_tail(st, xf_t, tt_s)
        pending_tail()

    nc.compile()
    return nc


_cache = {}


def _get_program(rows):
    if rows not in _cache:
        _cache[rows] = build_program(rows)
    return _cache[rows]


def _host_prep(weights, biases):
    beta_prefix = np.concatenate(
        [np.zeros((1, D), np.float32), np.cumsum(biases, axis=0)[:-1]], axis=0
    )  # beta_l = sum_{j<l} b_j
    cvec = np.sum(beta_prefix * weights, axis=1, dtype=np.float32)[None, :]  # [1, L]
    beta = np.sum(biases, axis=0, dtype=np.float32)[None, :]                 # [1, D]
    wt = np.ascontiguousarray(weights.T, dtype=np.float32)                   # [D, L]
    return wt, beta, cvec


def _aux_inputs():
    return np.eye(P, dtype=np.float32)


def kernel(x, weights, biases):
    x = np.ascontiguousarray(x, dtype=np.float32)
    weights = np.asarray(weights, dtype=np.float32)
    biases = np.asarray(biases, dtype=np.float32)

    wt, beta, cvec = _host_prep(weights, biases)
    iden = _aux_inputs()
    nc = _get_program(R)
    in_maps = [
        {"x": x[i * R:(i + 1) * R], "wt": wt, "beta": beta, "cvec": cvec,
         "iden": iden}
        for i in range(N_CORES)
    ]
    res = run_bass_kernel_spmd(nc, in_maps, list(range(N_CORES)))
    return np.concatenate([res.results[i]["out"] for i in range(N_CORES)], axis=0)


# revision 21
# speedup vs baseline: 1.0758x; 1.0758x over previous
"""CrossNetwork (DCN) kernel for 8 Trainium2 NeuronCores.

Math: the L=4 cross layers  x_{i+1} = x0 * (x_i . w_i) + b_i + x_i  collapse to
    out = alpha * x0 + beta
where beta = sum_l b_l, and per-row alpha follows the scalar recurrence
    t_l = x0 . w_l          (4 per-row dot products)
    u_l = 1 + t_l
    alpha = u_0;  alpha = alpha * u_l + c_l   (l = 1..3)
with c_l = (sum_{j<l} b_j) . w_l  (host-precomputed scalars).

The per-row dots run on the tensor engine in fp16: x is cast once on DVE, the
32 [128,128] chunks of each 512-row supertile are PE-transposed into PSUM,
bounced to SBUF by the scalar engine, then contracted against an augmented
[w_hi | w_lo] fp16 weight pair (so the weights contribute fp32-exact values)
in 8 accumulating matmuls -> T^T[8, 512]. Tiny PE transposes bring T^T back
to [rows, 8]; u = 1 + t_hi + t_lo fuses the halves; alpha comes from one
tensor_tensor_scan per row block; the finale is one fused DVE op per block:
out = (x_f32 * alpha) + beta.

The per-supertile tail (small transposes, scan, finale, store) is emitted one
supertile late so its PE ops never block the next supertile's transposes in
the in-order PE FIFO; stores trigger from the gpsimd queue to keep their
waits off the ACT/Sync queues.

Sharding: data-parallel over the batch dim, 4096 rows per core; the tiny
weights/biases-derived tensors are replicated.
"""

import sys

if "/opt/trn_rl_repo" not in sys.path:
    sys.path.insert(0, "/opt/trn_rl_repo")

from contextlib import ExitStack

import numpy as np

import concourse.bass as bass
import concourse.tile as tile
from concourse import bacc, mybir
from concourse.bass_utils import run_bass_kernel_spmd
from concourse.masks import make_identity

N_CORES = 8
B, D, L = 32768, 1024, 4
R = B // N_CORES          # rows per core
P = 128                   # partitions
SB = 4                    # row-blocks per supertile (512 rows)
NST = R // (SB * P)       # supertiles per core
NCH = D // P              # 128-col chunks per row
L2 = 2 * L                # augmented weight columns (hi + lo halves)
F16 = mybir.dt.float16
F32 = mybir.dt.float32


def build_program(rows=R):
    nst = rows // (SB * P)
    nc = bacc.Bacc("TRN2", target_bir_lowering=False, debug=False)
    x = nc.dram_tensor("x", [rows, D], F32, kind="ExternalInput")
    wt = nc.dram_tensor("wt", [D, L2], F16, kind="ExternalInput")
    iden = nc.dram_tensor("iden", [P, P], F16, kind="ExternalInput")
    beta = nc.dram_tensor("beta", [1, D], F32, kind="ExternalInput")
    cvec = nc.dram_tensor("cvec", [1, L], F32, kind="ExternalInput")
    out = nc.dram_tensor("out", [rows, D], F32, kind="ExternalOutput")

    xr = x.rearrange("(s b p) d -> s p b d", b=SB, p=P)
    outr = out.rearrange("(s b p) d -> s p b d", b=SB, p=P)

    with tile.TileContext(nc) as tc, ExitStack() as ctx:
        consts = ctx.enter_context(tc.tile_pool(name="consts", bufs=1))
        xf_pool = ctx.enter_context(tc.tile_pool(name="xf", bufs=6))
        xh_pool = ctx.enter_context(tc.tile_pool(name="xh", bufs=2))
        xbt_sb_pool = ctx.enter_context(tc.tile_pool(name="xbt_sb", bufs=10))
        small_pool = ctx.enter_context(tc.tile_pool(name="small", bufs=3))
        osb_pool = ctx.enter_context(tc.tile_pool(name="osb", bufs=3))
        xbt_ps_pool = ctx.enter_context(tc.tile_pool(name="xbt_ps", bufs=4, space="PSUM"))
        tt_ps_pool = ctx.enter_context(tc.tile_pool(name="tt_ps", bufs=2, space="PSUM"))
        t_ps_pool = ctx.enter_context(tc.tile_pool(name="t_ps", bufs=2, space="PSUM"))

        # constants
        ident = consts.tile([P, P], F16)
        nc.gpsimd.dma_start(out=ident, in_=iden[:])
        ident8 = consts.tile([L2, L2], F32)
        make_identity(nc, ident8)
        wt_sb = consts.tile([P, NCH, L2], F16)
        nc.gpsimd.dma_start(out=wt_sb, in_=wt.rearrange("(k p) m -> p k m", p=P))
        beta_sb = consts.tile([P, D], F32)
        beta_ap = beta[:]
        nc.gpsimd.dma_start(
            out=beta_sb,
            in_=bass.AP(
                tensor=beta_ap.tensor,
                offset=beta_ap.offset,
                ap=[[0, P], [1, D]],
            ),
        )
        c_sb = consts.tile([P, L], F32)
        cvec_ap = cvec[:]
        nc.gpsimd.dma_start(
            out=c_sb,
            in_=bass.AP(
                tensor=cvec_ap.tensor,
                offset=cvec_ap.offset,
                ap=[[0, P], [1, L]],
            ),
        )

        def make_tail(st, xf_t, tt_s):
            # Everything past the dot matmuls for supertile `st`; emitted one
            # supertile late (see module docstring).
            def tail():
                # transpose T^T back to [rows, hi/lo x l] per row-block
                t_p = t_ps_pool.tile([P, SB, L2], F32)
                for b in range(SB):
                    nc.tensor.transpose(
                        t_p[:, b], tt_s[:, b * P:(b + 1) * P], ident8
                    )
                # u = 1 + t_hi + t_lo (two steps: only one PSUM input per op)
                u0 = small_pool.tile([P, SB, L], F32, tag="u0")
                nc.scalar.activation(
                    u0, t_p[:, :, 0:L],
                    mybir.ActivationFunctionType.Identity, bias=1.0,
                )
                u_t = small_pool.tile([P, SB, L], F32, tag="u")
                nc.vector.tensor_tensor(
                    out=u_t, in0=u0, in1=t_p[:, :, L:L2], op=mybir.AluOpType.add
                )

                # per row block: alpha via one linear scan over layers
                # (state = u_0; state = state*u_l + c_l), then the fused
                # finale out = (x * alpha) + beta, then store per pair
                al = small_pool.tile([P, SB, L], F32, tag="al")
                for b in range(SB):
                    nc.vector.tensor_tensor_scan(
                        out=al[:, b],
                        data0=u_t[:, b],
                        data1=c_sb,
                        initial=1.0,
                        op0=mybir.AluOpType.mult,
                        op1=mybir.AluOpType.add,
                    )
                o_t = osb_pool.tile([P, SB, D], F32)
                for b in range(SB):
                    nc.vector.affine_then_add(
                        out=o_t[:, b],
                        in0=xf_t[:, b],
                        in1=beta_sb,
                        scale=al[:, b, L - 1:L],
                        bias=0.0,
                    )
                    if b % 2 == 1:
                        # gpsimd (SWDGE) trigger: keeps the store's wait on
                        # the DVE affines off the ACT and Sync queues
                        nc.gpsimd.dma_start(
                            out=outr[st][:, b - 1:b + 1],
                            in_=o_t[:, b - 1:b + 1],
                        )

            return tail

        pending_tail = None
        for st in range(nst):
            xf_t = xf_pool.tile([P, SB, D], F32)
            nc.sync.dma_start(out=xf_t, in_=xr[st])

            # fp16 working copy for the dot-product path (DVE 2x_2p)
            xh_t = xh_pool.tile([P, SB, D], F16)
            nc.vector.tensor_copy(xh_t, xf_t)

            # transpose all 32 [128,128] chunks, bouncing each column chunk
            # through PSUM to SBUF
            xbt_list = []
            for c in range(NCH):
                cs = slice(c * P, (c + 1) * P)
                xbt_p = xbt_ps_pool.tile([P, SB, P], F16)
                for b in range(SB):
                    nc.tensor.transpose(xbt_p[:, b], xh_t[:, b, cs], ident)
                xbt_s = xbt_sb_pool.tile([P, SB * P], F16)
                nc.scalar.copy(xbt_s, xbt_p)
                xbt_list.append(xbt_s)

            # 8 dot matmuls accumulating T^T[hi/lo x l, rows] over chunks
            tt_p = tt_ps_pool.tile([L2, SB * P], F32)
            for c in range(NCH):
                nc.tensor.matmul(
                    tt_p,
                    wt_sb[:, c],
                    xbt_list[c],
                    start=(c == 0),
                    stop=(c == NCH - 1),
                )

            tt_s = small_pool.tile([L2, SB * P], F32, tag="tt_s")
            nc.scalar.copy(tt_s, tt_p)

            if pending_tail is not None:
                pending_tail()
            pending_tail = make_tail(st, xf_t, tt_s)
        pending_tail()

    nc.compile()
    return nc


_cache = {}


def _get_program(rows):
    if rows not in _cache:
        _cache[rows] = build_program(rows)
    return _cache[rows]


def _host_prep(weights, biases):
    beta_prefix = np.concatenate(
        [np.zeros((1, D), np.float32), np.cumsum(biases, axis=0)[:-1]], axis=0
    )  # beta_l = sum_{j<l} b_j
    cvec = np.sum(beta_prefix * weights, axis=1, dtype=np.float32)[None, :]  # [1, L]
    beta = np.sum(biases, axis=0, dtype=np.float32)[None, :]                 # [1, D]
    # augmented fp16 weight pair: w ~= w_hi + w_lo to fp32 accuracy
    w_hi = weights.astype(np.float16)
    w_lo = (weights - w_hi.astype(np.float32)).astype(np.float16)
    wt = np.zeros((D, L2), dtype=np.float16)                                 # [D, 8]
    wt[:, :L] = w_hi.T
    wt[:, L:] = w_lo.T
    return wt, beta, cvec


def _aux_inputs():
    return np.eye(P, dtype=np.float16)


def kernel(x, weights, biases):
    x = np.ascontiguousarray(x, dtype=np.float32)
    weights = np.asarray(weights, dtype=np.float32)
    biases = np.asarray(biases, dtype=np.float32)

    wt, beta, cvec = _host_prep(weights, biases)
    iden = _aux_inputs()
    nc = _get_program(R)
    in_maps = [
        {"x": x[i * R:(i + 1) * R], "wt": wt, "beta": beta, "cvec": cvec,
         "iden": iden}
        for i in range(N_CORES)
    ]
    res = run_bass_kernel_spmd(nc, in_maps, list(range(N_CORES)))
    return np.concatenate([res.results[i]["out"] for i in range(N_CORES)], axis=0)
